# revision 18
# baseline (speedup 1.0000x reference)
"""Trainium2 Bass kernel for nn_Evolution (deep-snake polygon evolution).

kernel(**inputs) takes the FULL unsharded inputs, shards instances across
8 NeuronCores (one image pair per core, capacity-padded), runs one SPMD
Bass program (grid-sample via dma_gather + 4 snake CNNs with cross-core
BatchNorm via AllGather), and reassembles full outputs.
"""
import sys, os
sys.path.insert(0, "/opt/trn_rl_repo")

import numpy as np
import ml_dtypes
from contextlib import ExitStack

import concourse.bass as bass
import concourse.tile as tile
from concourse import bacc, mybir
from concourse.bass_utils import run_bass_kernel_spmd
from concourse.masks import make_identity

BF16 = ml_dtypes.bfloat16
F32 = mybir.dt.float32
BF = mybir.dt.bfloat16
I16 = mybir.dt.int16
I32 = mybir.dt.int32
AL = mybir.AluOpType
AF = mybir.ActivationFunctionType
AX = mybir.AxisListType

NCORES = 8
RO = 4.0
DILS = [1, 1, 1, 1, 2, 2, 4, 4]   # head + 7 res layers
PAD = 16
IMGROWS = 128 * 128
EPS = 1e-5

LOCAL_BN = os.environ.get("NN_EVO_LOCAL_BN", "0") == "1"

_cache = {}


def _ap(t, offset, dims):
    return bass.AP(tensor=t.tensor, offset=t.offset + offset, ap=dims)


def _binner(a, n):
    """append a 0-step innermost broadcast dim to an AP"""
    return bass.AP(tensor=a.tensor, offset=a.offset, ap=a.ap + [[0, n]])


# ============================================================================
# program builder
# ============================================================================

def _build(M, local_bn):
    NPTS = 128 * M
    NP4 = 40 * M
    L4G = -(-(41 * M) // 128) * 128
    chE = [(4 * i, 4) for i in range(M // 4)]
    ch4 = []
    s = 0
    while s < M:
        n = min(12, M - s)
        ch4.append((s, n))
        s += n
    DUMN_E = float((NCORES * M - 256) * 128)
    DUMN_I = float((NCORES * M - 256) * 40)
    INV_E = 1.0 / (256 * 128)
    INV_I = 1.0 / (256 * 40)

    nc = bacc.Bacc("TRN2", target_bir_lowering=False, debug=False,
                   num_devices=NCORES)

    def din(name, shape, dt):
        return nc.dram_tensor(name, shape, dt, kind="ExternalInput")

    tab = din("tab", [2 * IMGROWS + 1, 128], BF)
    cin1 = din("cin1", [2, NPTS], BF)
    cin4 = din("cin4", [2, NP4], BF)
    base1 = din("base1", [2, NPTS], F32)
    dmask_in = din("dmask_t", [128, M], F32)
    sbase_in = din("sbase_t", [128, M], F32)
    idx1a = din("idx1a", [128, NPTS // 16], I16)
    idx1b = din("idx1b", [128, NPTS // 16], I16)
    w1aa = din("w1aa", [128, NPTS], BF)
    w1bb = din("w1bb", [128, NPTS], BF)
    idx4a = din("idx4a", [128, L4G // 16], I16)
    idx4b = din("idx4b", [128, L4G // 16], I16)
    w4aa = din("w4aa", [128, L4G], BF)
    w4bb = din("w4bb", [128, L4G], BF)
    locstat = din("locstat", [128, 2], F32)
    locdum = din("locdum", [128, 2], F32)
    wfu_in = din("wfu", [128, 64], BF)
    bfu_in = din("bfu", [64, 1], F32)

    snames = ["sI", "s1", "s2", "s3"]
    P = {}
    for sn in snames:
        for l in range(8):
            cin = 66 if l == 0 else 128
            P[f'{sn}_wc{l}'] = din(f'{sn}_wc{l}', [cin, 9, 128], BF)
            P[f'{sn}_ws{l}'] = din(f'{sn}_ws{l}', [cin, 128], BF)
            P[f'{sn}_cb{l}'] = din(f'{sn}_cb{l}', [128, 1], F32)
            P[f'{sn}_bn{l}'] = din(f'{sn}_bn{l}', [128, 2], F32)
        P[f'{sn}_wf'] = din(f'{sn}_wf', [128, 8, 256], BF)
        P[f'{sn}_bf'] = din(f'{sn}_bf', [128, 2], F32)
        P[f'{sn}_w0'] = din(f'{sn}_w0', [128, 10, 256], BF)
        P[f'{sn}_b0'] = din(f'{sn}_b0', [128, 2], F32)
        P[f'{sn}_w1'] = din(f'{sn}_w1', [128, 2, 64], BF)
        P[f'{sn}_b1'] = din(f'{sn}_b1', [64, 1], F32)
        P[f'{sn}_w2'] = din(f'{sn}_w2', [64, 2], BF)
        P[f'{sn}_b2'] = din(f'{sn}_b2', [2, 1], F32)

    d4_out = nc.dram_tensor("d4", [2, NP4], F32, kind="ExternalOutput")
    py_out = [nc.dram_tensor(f"py{r}", [2, NPTS], F32, kind="ExternalOutput")
              for r in (1, 2, 3)]

    with tile.TileContext(nc) as tc, ExitStack() as ctx:
        sing = ctx.enter_context(tc.tile_pool(name="sing", bufs=1))
        states = ctx.enter_context(tc.tile_pool(name="states", bufs=8))
        xpads = ctx.enter_context(tc.tile_pool(name="xpads", bufs=1))
        st4 = ctx.enter_context(tc.tile_pool(name="st4", bufs=8))
        xp4 = ctx.enter_context(tc.tile_pool(name="xp4", bufs=2))
        zp = ctx.enter_context(tc.tile_pool(name="zp", bufs=1))
        gp = ctx.enter_context(tc.tile_pool(name="gp", bufs=2))
        wpl = ctx.enter_context(tc.tile_pool(name="wpl", bufs=1))
        wcp = ctx.enter_context(tc.tile_pool(name="wcp", bufs=2))
        bigw = ctx.enter_context(tc.tile_pool(name="bigw", bufs=1))
        smp = ctx.enter_context(tc.tile_pool(name="smp", bufs=4))
        cmp_ = ctx.enter_context(tc.tile_pool(name="cmp", bufs=14))
        wtp = ctx.enter_context(tc.tile_pool(name="wtp", bufs=1))
        chp = ctx.enter_context(tc.tile_pool(name="chp", bufs=4))
        pyp = ctx.enter_context(tc.tile_pool(name="pyp", bufs=2))
        psc = ctx.enter_context(tc.tile_pool(name="psc", bufs=4, space="PSUM"))
        pst = ctx.enter_context(tc.tile_pool(name="pst", bufs=2, space="PSUM"))
        pss = ctx.enter_context(tc.tile_pool(name="pss", bufs=2, space="PSUM"))
        dram = ctx.enter_context(tc.tile_pool(name="dram", bufs=2, space="DRAM"))

        idf = sing.tile([128, 128], F32, tag="idf")
        make_identity(nc, idf[:])
        eps_t = sing.tile([128, 1], F32, tag="eps")
        nc.vector.memset(eps_t[:], EPS)
        dmask_s = sing.tile([128, M], F32, tag="dmk")
        nc.sync.dma_start(dmask_s[:], dmask_in[:])
        sbase_s = sing.tile([128, M], F32, tag="sbs")
        nc.sync.dma_start(sbase_s[:], sbase_in[:])
        locs_s = sing.tile([128, 2], F32, tag="lcs")
        nc.sync.dma_start(locs_s[:], locstat[:])
        locd_s = sing.tile([128, 2], F32, tag="lcd")
        nc.sync.dma_start(locd_s[:], locdum[:])

        # -------------------------------------------------------------- util
        def gather_combine(idxa, idxb, load_a, load_b, L, segs, GMAX,
                           main_out, main_len, ct_out=None, ct_cols=None):
            """segs: list of (lo, hi) flat col ranges (hi-lo mult of 128).
            main_out(lo, hi) -> AP for feat cols [lo, hi).
            """
            for (lo, hi) in segs:
                sl = hi - lo
                g0 = gp.tile([128, 1, GMAX], BF, tag="g", name=f"g0_{lo}")
                nc.gpsimd.dma_gather(
                    out_ap=g0[:, :, 0:sl], in_ap=tab.ap(),
                    idxs_ap=idxa[:, lo // 16:hi // 16],
                    num_idxs=sl, num_idxs_reg=sl, elem_size=128,
                    transpose=True, single_packet=False)
                wt = wpl.tile([128, GMAX], BF, tag="w", name=f"wt_{lo}")
                load_a(wt, lo, hi)
                nc.vector.tensor_tensor(out=g0[:, 0, 0:sl],
                                        in0=g0[:, 0, 0:sl],
                                        in1=wt[:, 0:sl], op=AL.mult)
                g1 = gp.tile([128, 1, GMAX], BF, tag="g", name=f"g1_{lo}")
                nc.gpsimd.dma_gather(
                    out_ap=g1[:, :, 0:sl], in_ap=tab.ap(),
                    idxs_ap=idxb[:, lo // 16:hi // 16],
                    num_idxs=sl, num_idxs_reg=sl, elem_size=128,
                    transpose=True, single_packet=False)
                wt2 = wpl.tile([128, GMAX], BF, tag="w", name=f"wu_{lo}")
                load_b(wt2, lo, hi)
                nc.vector.tensor_tensor(out=g1[:, 0, 0:sl],
                                        in0=g1[:, 0, 0:sl],
                                        in1=wt2[:, 0:sl], op=AL.mult)
                nc.vector.tensor_tensor(out=g0[:, 0, 0:sl],
                                        in0=g0[:, 0, 0:sl],
                                        in1=g1[:, 0, 0:sl], op=AL.add)
                fld = wpl.tile([64, GMAX], BF, tag="w", name=f"fld_{lo}")
                nc.vector.tensor_copy(fld[:, 0:sl], g0[64:128, 0, 0:sl])
                mlo, mhi = lo, min(hi, main_len)
                if mhi > mlo:
                    nc.vector.tensor_tensor(
                        out=main_out(mlo, mhi),
                        in0=g0[0:64, 0, 0:mhi - mlo],
                        in1=fld[:, 0:mhi - mlo], op=AL.add)
                if ct_out is not None:
                    c0, c1 = ct_cols
                    a, b = max(lo, c0), min(hi, c1)
                    if b > a:
                        nc.vector.tensor_tensor(
                            out=ct_out[:, a - c0:b - c0],
                            in0=g0[0:64, 0, a - lo:b - lo],
                            in1=fld[:, a - lo:b - lo], op=AL.add)

        def wraps(pad, rows, PTS):
            nc.vector.tensor_copy(pad[0:rows, :, 0:PAD],
                                  pad[0:rows, :, PTS:PTS + PAD])
            nc.vector.tensor_copy(pad[0:rows, :, PAD + PTS:2 * PAD + PTS],
                                  pad[0:rows, :, PAD:2 * PAD])

        sres = {}

        # --------------------------------------------------- snake conv body
        def snake_layers(sn, x0pad, PTS, PW, chunks, isinit):
            spool, ptag = (xp4, "pad4x") if isinit else (states, "pad")
            prev_body = None
            sres[sn] = []
            vcf = smp.tile([128, 1], F32, tag="vc")
            nc.vector.memset(vcf[:], 0.0)
            vcb = smp.tile([128, 1], BF, tag="vcb")
            nc.vector.memset(vcb[:], 0.0)
            xpad = x0pad
            dumn = DUMN_I if isinit else DUMN_E
            invn = INV_I if isinit else INV_E
            npts = PTS * M
            for l in range(8):
                cinn = 66 if l == 0 else 128
                d = DILS[l]
                wc = wcp.tile([cinn, 9, 128], BF, tag="wc")
                nc.sync.dma_start(wc[:], P[f'{sn}_wc{l}'][:])
                ws = wcp.tile([cinn, 128], BF, tag="ws")
                nc.sync.dma_start(ws[:], P[f'{sn}_ws{l}'][:])
                cb = smp.tile([128, 1], F32, tag="cb")
                nc.sync.dma_start(cb[:], P[f'{sn}_cb{l}'][:])
                bnp = smp.tile([128, 2], F32, tag="bnp")
                nc.sync.dma_start(bnp[:], P[f'{sn}_bn{l}'][:])
                z = zp.tile([128, npts], BF, tag="z4" if isinit else "zE",
                            name=f"z_{sn}{l}")
                stt = smp.tile([128, len(chunks), 6], F32, tag="stt")
                for ci, (s0, nin) in enumerate(chunks):
                    fr = nin * PTS
                    ps = psc.tile([128, 512], F32, tag="ps")
                    for k in range(9):
                        o = PAD + (k - 4) * d
                        nc.tensor.matmul(
                            ps[:, 0:fr], wc[:, k, :],
                            xpad[0:cinn, s0:s0 + nin, o:o + PTS],
                            start=(k == 0), stop=(k == 8))
                    zc = z[:, s0 * PTS:s0 * PTS + fr]
                    nc.scalar.activation(out=zc, in_=ps[:, 0:fr], func=AF.Relu,
                                         bias=cb[:], scale=1.0)
                    nc.vector.bn_stats(out=stt[:, ci, :], in_=zc)
                mv = smp.tile([128, 2], F32, tag="mv")
                nc.vector.bn_aggr(out=mv[:], in_=stt[:])
                # dummy-column value tracker
                psv = pss.tile([128, 16], F32, tag="psv")
                nc.tensor.matmul(psv[:, 0:1], ws[:], vcb[0:cinn, :],
                                 start=True, stop=True)
                vz = smp.tile([128, 1], F32, tag="vz")
                nc.scalar.activation(out=vz[:], in_=psv[:, 0:1], func=AF.Relu,
                                     bias=cb[:], scale=1.0)
                vz2 = smp.tile([128, 1], F32, tag="vz2")
                nc.vector.tensor_tensor(out=vz2[:], in0=vz[:], in1=vz[:],
                                        op=AL.mult)
                sums = smp.tile([128, 2], F32, tag="sums")
                nc.vector.tensor_scalar(out=sums[:, 0:1], in0=mv[:, 0:1],
                                        scalar1=float(npts), scalar2=None,
                                        op0=AL.mult)
                m2 = smp.tile([128, 1], F32, tag="m2")
                nc.vector.tensor_tensor(out=m2[:], in0=mv[:, 0:1],
                                        in1=mv[:, 0:1], op=AL.mult)
                nc.vector.tensor_tensor(out=sums[:, 1:2], in0=mv[:, 1:2],
                                        in1=m2[:], op=AL.add)
                nc.vector.tensor_scalar(out=sums[:, 1:2], in0=sums[:, 1:2],
                                        scalar1=float(npts), scalar2=None,
                                        op0=AL.mult)
                corr = smp.tile([128, 2], F32, tag="corr")
                if not local_bn:
                    cci = dram.tile([128, 2], F32, tag="cci")
                    cco = dram.tile([NCORES * 128, 2], F32, tag="cco",
                                    addr_space="Shared")
                    nc.sync.dma_start(cci[:], sums[:])
                    nc.gpsimd.collective_compute(
                        "AllGather", AL.bypass,
                        replica_groups=[list(range(NCORES))],
                        ins=[cci[:]], outs=[cco[:]])
                    gat = smp.tile([128, 16], F32, tag="gat")
                    nc.sync.dma_start(
                        gat[:], _ap(cco, 0, [[2, 128], [256, 8], [1, 2]]))
                    r4 = smp.tile([128, 8], F32, tag="r4")
                    nc.vector.tensor_tensor(out=r4[:], in0=gat[:, 0:8],
                                            in1=gat[:, 8:16], op=AL.add)
                    nc.vector.tensor_tensor(out=r4[:, 0:4], in0=r4[:, 0:4],
                                            in1=r4[:, 4:8], op=AL.add)
                    nc.vector.tensor_tensor(out=sums[:], in0=r4[:, 0:2],
                                            in1=r4[:, 2:4], op=AL.add)
                    nc.vector.tensor_scalar(out=corr[:, 0:1], in0=vz[:],
                                            scalar1=dumn, scalar2=None,
                                            op0=AL.mult)
                    nc.vector.tensor_scalar(out=corr[:, 1:2], in0=vz2[:],
                                            scalar1=dumn, scalar2=None,
                                            op0=AL.mult)
                    nc.vector.tensor_tensor(out=sums[:], in0=sums[:],
                                            in1=corr[:], op=AL.subtract)
                    mean = smp.tile([128, 1], F32, tag="mean")
                    nc.vector.tensor_scalar(out=mean[:], in0=sums[:, 0:1],
                                            scalar1=invn, scalar2=None,
                                            op0=AL.mult)
                    var = smp.tile([128, 1], F32, tag="var")
                    nc.vector.tensor_scalar(out=var[:], in0=sums[:, 1:2],
                                            scalar1=invn, scalar2=None,
                                            op0=AL.mult)
                else:
                    lc = 1 if isinit else 0
                    nc.vector.tensor_scalar(out=corr[:, 0:1], in0=vz[:],
                                            scalar1=locd_s[:, lc:lc + 1],
                                            scalar2=None, op0=AL.mult)
                    nc.vector.tensor_scalar(out=corr[:, 1:2], in0=vz2[:],
                                            scalar1=locd_s[:, lc:lc + 1],
                                            scalar2=None, op0=AL.mult)
                    nc.vector.tensor_tensor(out=sums[:], in0=sums[:],
                                            in1=corr[:], op=AL.subtract)
                    mean = smp.tile([128, 1], F32, tag="mean")
                    nc.vector.tensor_scalar(out=mean[:], in0=sums[:, 0:1],
                                            scalar1=locs_s[:, lc:lc + 1],
                                            scalar2=None, op0=AL.mult)
                    var = smp.tile([128, 1], F32, tag="var")
                    nc.vector.tensor_scalar(out=var[:], in0=sums[:, 1:2],
                                            scalar1=locs_s[:, lc:lc + 1],
                                            scalar2=None, op0=AL.mult)
                m2b = smp.tile([128, 1], F32, tag="m2b")
                nc.vector.tensor_tensor(out=m2b[:], in0=mean[:], in1=mean[:],
                                        op=AL.mult)
                nc.vector.tensor_tensor(out=var[:], in0=var[:], in1=m2b[:],
                                        op=AL.subtract)
                rs = smp.tile([128, 1], F32, tag="rs")
                nc.scalar.activation(out=rs[:], in_=var[:], func=AF.Sqrt,
                                     bias=eps_t[:], scale=1.0)
                nc.vector.reciprocal(out=rs[:], in_=rs[:])
                av = smp.tile([128, 1], F32, tag="av")
                nc.vector.tensor_tensor(out=av[:], in0=rs[:],
                                        in1=bnp[:, 0:1], op=AL.mult)
                cv = smp.tile([128, 1], F32, tag="cv")
                nc.vector.tensor_tensor(out=cv[:], in0=mean[:], in1=av[:],
                                        op=AL.mult)
                nc.vector.tensor_tensor(out=cv[:], in0=bnp[:, 1:2], in1=cv[:],
                                        op=AL.subtract)
                vn = smp.tile([128, 1], F32, tag="vc")
                nc.vector.tensor_scalar(out=vn[:], in0=vz[:], scalar1=av[:],
                                        scalar2=cv[:], op0=AL.mult,
                                        op1=AL.add)
                if l > 0:
                    nc.vector.tensor_tensor(out=vn[:], in0=vn[:], in1=vcf[:],
                                            op=AL.add)
                vcf = vn
                vcb = smp.tile([128, 1], BF, tag="vcb")
                nc.vector.tensor_copy(vcb[:], vcf[:])
                if isinit:
                    body = st4.tile([128, M, PTS], BF, tag="bod4",
                                    name=f"bod4_{l}")
                else:
                    body = None
                npad = spool.tile([128, M, PW], BF, tag=ptag,
                                  name=f"pad_{sn}{l}")
                for (s0, nin) in chunks:
                    fr = nin * PTS
                    zc = z[:, s0 * PTS:s0 * PTS + fr]
                    if isinit:
                        bodyc = body[:, s0:s0 + nin, :]
                        resc = (None if l == 0
                                else prev_body[:, s0:s0 + nin, :])
                    else:
                        bodyc = npad[:, s0:s0 + nin, PAD:PAD + PTS]
                        resc = (None if l == 0
                                else xpad[:, s0:s0 + nin, PAD:PAD + PTS])
                    if l == 0:
                        nc.vector.tensor_scalar(out=bodyc, in0=zc,
                                                scalar1=av[:], scalar2=cv[:],
                                                op0=AL.mult, op1=AL.add)
                    else:
                        tmpc = chp.tile([128, 512], BF, tag="tmpc")
                        nc.vector.tensor_scalar(out=tmpc[:, 0:fr], in0=zc,
                                                scalar1=av[:], scalar2=cv[:],
                                                op0=AL.mult, op1=AL.add)
                        nc.vector.tensor_tensor(out=bodyc, in0=tmpc[:, 0:fr],
                                                in1=resc, op=AL.add)
                if isinit:
                    nc.vector.tensor_copy(npad[:, :, PAD:PAD + PTS], body[:])
                    prev_body = body
                    sres[sn].append(body)
                else:
                    sres[sn].append(npad)
                wraps(npad, 128, PTS)
                xpad = npad
                yield

        # ------------------------------------------------------- snake tail
        def snake_tail(sn, PTS, chunks, base_src, out_dram, stage=None,
                       off=PAD):
            sts = sres[sn]
            wf = bigw.tile([128, 8, 256], BF, tag="wf")
            nc.sync.dma_start(wf[:], P[f'{sn}_wf'][:])
            bf_ = smp.tile([128, 2], F32, tag="bf_")
            nc.sync.dma_start(bf_[:], P[f'{sn}_bf'][:])
            gmax = []
            for mt in range(2):
                gx = sing.tile([128, M], BF, tag=f"gx{sn}{mt}",
                               name=f"gx{sn}{mt}")
                gmax.append(gx)
            for mt in range(2):
                for gs in range(0, len(chunks), 4):
                    grp = chunks[gs:gs + 4]
                    pls = []
                    for _g in grp:
                        pstile = psc.tile([128, 512], F32, tag="ps")
                        pls.append(pstile)
                    for kb in range(8):
                        for gi, (s0, nin) in enumerate(grp):
                            nc.tensor.matmul(
                                pls[gi][:, 0:nin * PTS],
                                wf[:, kb, 128 * mt:128 * (mt + 1)],
                                sts[kb][:, s0:s0 + nin, off:off + PTS],
                                start=(kb == 0), stop=(kb == 7))
                    for gi, (s0, nin) in enumerate(grp):
                        gm = smp.tile([128, 12], F32, tag="gmf")
                        nc.vector.tensor_reduce(
                            out=gm[:, 0:nin],
                            in_=pls[gi][:, 0:nin * PTS].rearrange(
                                "p (a b) -> p a b", a=nin),
                            op=AL.max, axis=AX.X)
                        nc.vector.tensor_scalar(
                            out=gmax[mt][:, s0:s0 + nin], in0=gm[:, 0:nin],
                            scalar1=bf_[:, mt:mt + 1], scalar2=None,
                            op0=AL.add)
            w0 = bigw.tile([128, 10, 256], BF, tag="w0")
            nc.sync.dma_start(w0[:], P[f'{sn}_w0'][:])
            b0 = smp.tile([128, 2], F32, tag="b0")
            nc.sync.dma_start(b0[:], P[f'{sn}_b0'][:])
            w1 = bigw.tile([128, 2, 64], BF, tag="w1")
            nc.sync.dma_start(w1[:], P[f'{sn}_w1'][:])
            b1 = smp.tile([64, 1], F32, tag="b1")
            nc.sync.dma_start(b1[:], P[f'{sn}_b1'][:])
            w2 = bigw.tile([64, 2], BF, tag="w2")
            nc.sync.dma_start(w2[:], P[f'{sn}_w2'][:])
            b2 = smp.tile([2, 1], F32, tag="b2")
            nc.sync.dma_start(b2[:], P[f'{sn}_b2'][:])
            for gs in range(0, len(chunks), 2):
                grp = chunks[gs:gs + 2]
                z0c = {}
                for mt in range(2):
                    pls = []
                    for _g in grp:
                        pstile = psc.tile([128, 512], F32, tag="ps")
                        pls.append(pstile)
                    for kb in range(10):
                        for gi, (s0, nin) in enumerate(grp):
                            fr = nin * PTS
                            if kb < 2:
                                rhs = _binner(gmax[kb][:, s0:s0 + nin], PTS)
                                nc.tensor.matmul(
                                    pls[gi][:, 0:fr],
                                    w0[:, kb, 128 * mt:128 * (mt + 1)], rhs,
                                    start=(kb == 0), stop=False)
                            else:
                                nc.tensor.matmul(
                                    pls[gi][:, 0:fr],
                                    w0[:, kb, 128 * mt:128 * (mt + 1)],
                                    sts[kb - 2][:, s0:s0 + nin,
                                                off:off + PTS],
                                    start=False, stop=(kb == 9))
                    for gi in range(len(grp)):
                        s0, nin = grp[gi]
                        zc = chp.tile([128, 512], BF, tag="z0c")
                        nc.scalar.activation(out=zc[:, 0:nin * PTS],
                                             in_=pls[gi][:, 0:nin * PTS],
                                             func=AF.Relu,
                                             bias=b0[:, mt:mt + 1], scale=1.0)
                        z0c[(mt, gi)] = zc
                for gi, (s0, nin) in enumerate(grp):
                    fr = nin * PTS
                    ph = psc.tile([128, 512], F32, tag="ps")
                    for mt in range(2):
                        nc.tensor.matmul(ph[0:64, 0:fr], w1[:, mt, :],
                                         z0c[(mt, gi)][:, 0:fr],
                                         start=(mt == 0), stop=(mt == 1))
                    h1 = chp.tile([64, 512], BF, tag="h1c")
                    nc.scalar.activation(out=h1[:, 0:fr], in_=ph[0:64, 0:fr],
                                         func=AF.Relu, bias=b1[:], scale=1.0)
                    p2 = psc.tile([128, 512], F32, tag="ps")
                    nc.tensor.matmul(p2[0:2, 0:fr], w2[:], h1[:, 0:fr],
                                     start=True, stop=True)
                    pyc = pyp.tile([2, 512], F32, tag="pyb", name="pyoc")
                    nc.scalar.activation(out=pyc[:, 0:fr], in_=p2[0:2, 0:fr],
                                         func=AF.Identity, bias=b2[:],
                                         scale=1.0)
                    c0 = s0 * PTS
                    if base_src is not None:
                        bsc = pyp.tile([2, 512], F32, tag="pyb")
                        nc.sync.dma_start(bsc[:, 0:fr],
                                          base_src[:, c0:c0 + fr])
                        nc.vector.tensor_tensor(out=pyc[:, 0:fr],
                                                in0=pyc[:, 0:fr],
                                                in1=bsc[:, 0:fr], op=AL.add)
                    nc.sync.dma_start(out_dram[:, c0:c0 + fr], pyc[:, 0:fr])
                    if stage is not None:
                        nc.sync.dma_start(stage[:, c0:c0 + fr], pyc[:, 0:fr])

        # --------------------------------------- device coord round (2 / 3)
        def coord_round(py_prev, x0pad, rtag):
            psT = pst.tile([128, 128], F32, tag="ptr")
            for c in range(M // 4):
                pyc = pyp.tile([2, 512], F32, tag="pyb", name="pylc")
                nc.sync.dma_start(pyc[:], py_prev[:, 512 * c:512 * (c + 1)])
                py3 = pyc[:].rearrange("p (a b) -> p a b", a=4)
                mnc = smp.tile([2, 4], F32, tag="mnc")
                nc.vector.tensor_reduce(out=mnc[:], in_=py3, op=AL.min,
                                        axis=AX.X)
                nc.vector.tensor_tensor(
                    out=x0pad[64:66, 4 * c:4 * c + 4, PAD:PAD + 128],
                    in0=py3, in1=_binner(mnc[:], 128), op=AL.subtract)
                for t in range(4):
                    nc.tensor.transpose(
                        psT[:, 2 * (4 * c + t):2 * (4 * c + t) + 2],
                        pyc[:, 128 * t:128 * (t + 1)], idf[0:2, 0:2])
            xy = sing.tile([128, 2 * M], F32, tag=f"xy{rtag}")
            nc.scalar.activation(out=xy[:], in_=psT[:, 0:2 * M], func=AF.Copy,
                                 scale=1.0)

            cmn = [0]

            def cm():
                cmn[0] += 1
                t = cmp_.tile([128, M], F32, tag="cm",
                              name=f"cm{rtag}_{cmn[0]}")
                return t

            res = {}
            for ax in (0, 1):
                src = _ap(xy, ax, [xy.ap[0], [2, M]])
                T = cm()
                nc.vector.tensor_scalar(out=T[:], in0=src, scalar1=0.25,
                                        scalar2=0.5, op0=AL.mult, op1=AL.add)
                XI = cmp_.tile([128, M], I32, tag="cmi", name=f"cmi{rtag}{ax}")
                nc.vector.tensor_copy(XI[:], T[:])
                XF = cm()
                nc.vector.tensor_copy(XF[:], XI[:])
                CG = cm()
                nc.vector.tensor_tensor(out=CG[:], in0=XF[:], in1=T[:],
                                        op=AL.is_gt)
                B = cm()
                nc.vector.tensor_tensor(out=B[:], in0=XF[:], in1=CG[:],
                                        op=AL.subtract)
                Wf_ = cm()
                nc.vector.tensor_tensor(out=Wf_[:], in0=T[:], in1=B[:],
                                        op=AL.subtract)
                V0 = cm()
                nc.vector.tensor_scalar(out=V0[:], in0=B[:], scalar1=0.5,
                                        scalar2=None, op0=AL.is_ge)
                V0b = cm()
                nc.vector.tensor_scalar(out=V0b[:], in0=B[:], scalar1=128.5,
                                        scalar2=None, op0=AL.is_le)
                nc.vector.tensor_tensor(out=V0[:], in0=V0[:], in1=V0b[:],
                                        op=AL.mult)
                V1 = cm()
                nc.vector.tensor_scalar(out=V1[:], in0=B[:], scalar1=127.5,
                                        scalar2=None, op0=AL.is_le)
                res[ax] = (B, Wf_, V0, V1)
            BX, WX, VX0, VX1 = res[0]
            BY, WY, VY0, VY1 = res[1]
            W0x = cm()
            nc.vector.tensor_scalar(out=W0x[:], in0=WX[:], scalar1=-1.0,
                                    scalar2=1.0, op0=AL.mult, op1=AL.add)
            nc.vector.tensor_tensor(out=W0x[:], in0=W0x[:], in1=VX0[:],
                                    op=AL.mult)
            W1x = cm()
            nc.vector.tensor_tensor(out=W1x[:], in0=WX[:], in1=VX1[:],
                                    op=AL.mult)
            W0y = cm()
            nc.vector.tensor_scalar(out=W0y[:], in0=WY[:], scalar1=-1.0,
                                    scalar2=1.0, op0=AL.mult, op1=AL.add)
            nc.vector.tensor_tensor(out=W0y[:], in0=W0y[:], in1=VY0[:],
                                    op=AL.mult)
            nc.vector.tensor_tensor(out=W0y[:], in0=W0y[:], in1=dmask_s[:],
                                    op=AL.mult)
            W1y = cm()
            nc.vector.tensor_tensor(out=W1y[:], in0=WY[:], in1=VY1[:],
                                    op=AL.mult)
            nc.vector.tensor_tensor(out=W1y[:], in0=W1y[:], in1=dmask_s[:],
                                    op=AL.mult)
            wlin = dram.tile([4, NPTS], BF, tag="wlin")
            for j, (wxp, wyp) in enumerate(
                    [(W0x, W0y), (W1x, W0y), (W0x, W1y), (W1x, W1y)]):
                wj = cm()
                nc.vector.tensor_tensor(out=wj[:], in0=wxp[:], in1=wyp[:],
                                        op=AL.mult)
                pw = pst.tile([128, 128], F32, tag="ptr")
                nc.tensor.transpose(pw[0:M, :], wj[:], idf[:])
                wT = wtp.tile([M, 128], F32, tag="wT")
                nc.scalar.activation(out=wT[:], in_=pw[0:M, :], func=AF.Copy,
                                     scale=1.0)
                nc.gpsimd.dma_start(
                    _ap(wlin, j * NPTS, [[128, M], [1, 128]]), wT[:])
            # row indices
            Ra = cm()
            nc.vector.tensor_scalar(out=Ra[:], in0=BY[:], scalar1=128.0,
                                    scalar2=None, op0=AL.mult)
            nc.vector.tensor_tensor(out=Ra[:], in0=Ra[:], in1=BX[:],
                                    op=AL.add)
            ilin = dram.tile([2, NPTS], I16, tag="ilin")
            iws = []
            for j, off in enumerate((-129.0, -1.0)):
                I_ = cm()
                nc.vector.tensor_scalar(out=I_[:], in0=Ra[:], scalar1=off,
                                        scalar2=None, op0=AL.add)
                nc.vector.tensor_scalar(out=I_[:], in0=I_[:], scalar1=0.0,
                                        scalar2=16383.0, op0=AL.max,
                                        op1=AL.min)
                nc.vector.tensor_tensor(out=I_[:], in0=I_[:], in1=sbase_s[:],
                                        op=AL.add)
                ii = cmp_.tile([128, M], I16, tag="cmi16", name=f"ci16_{rtag}{j}")
                nc.vector.tensor_copy(ii[:], I_[:])
                nc.sync.dma_start(
                    _ap(ilin, j * NPTS, [[1, 128], [128, M]]), ii[:])
                iw = sing.tile([128, NPTS // 16], I16, tag=f"iw{rtag}{j}")
                for ggi in range(8):
                    nc.sync.dma_start(
                        iw[16 * ggi:16 * ggi + 16, :],
                        _ap(ilin, j * NPTS, [[1, 16], [16, NPTS // 16]]))
                iws.append(iw)

            def mkload(j):
                def load(wt, lo, hi):
                    nc.sync.dma_start(
                        wt[0:64, 0:hi - lo],
                        _ap(wlin, j * NPTS + lo, [[0, 64], [1, hi - lo]]))
                    nc.sync.dma_start(
                        wt[64:128, 0:hi - lo],
                        _ap(wlin, (j + 1) * NPTS + lo,
                            [[0, 64], [1, hi - lo]]))
                return load

            def mo(lo, hi):
                return x0pad[0:64, lo // 128:hi // 128, PAD:PAD + 128]
            gather_combine(iws[0][:], iws[1][:], mkload(0), mkload(2), NPTS,
                           segE, GMAX, mo, NPTS)
            wraps(x0pad, 66, 128)

        # ===================================================== emission plan
        # round 1 prep
        x0p1 = xpads.tile([128, M, 160], BF, tag="padx")
        i1a = sing.tile([128, NPTS // 16], I16, tag="i1a")
        nc.sync.dma_start(i1a[:], idx1a[:])
        i1b = sing.tile([128, NPTS // 16], I16, tag="i1b")
        nc.sync.dma_start(i1b[:], idx1b[:])

        def _ld(src):
            def load(wt, lo, hi):
                nc.sync.dma_start(wt[:, 0:hi - lo], src[:, lo:hi])
            return load
        Q = NPTS // 4
        segE = [(i * Q, (i + 1) * Q) for i in range(4)]
        sp4_ = (40 * M) // 640 * 640
        GMAX = max(Q, 640, L4G - sp4_)

        def _mo(pad):
            def mo(lo, hi):
                return pad[0:64, lo // 128:hi // 128, PAD:PAD + 128]
            return mo
        gather_combine(i1a[:], i1b[:], _ld(w1aa), _ld(w1bb), NPTS, segE, GMAX,
                       _mo(x0p1), NPTS)
        nc.sync.dma_start(x0p1[64:66, :, PAD:PAD + 128], cin1[:])
        wraps(x0p1, 66, 128)

        # init prep
        i4a = sing.tile([128, L4G // 16], I16, tag="i4a")
        nc.sync.dma_start(i4a[:], idx4a[:])
        i4b = sing.tile([128, L4G // 16], I16, tag="i4b")
        nc.sync.dma_start(i4b[:], idx4b[:])
        ff = sing.tile([128, M, 40], BF, tag="ff")
        ct = sing.tile([64, M], BF, tag="ct")
        sp4 = sp4_
        seg4 = [(x, min(x + 640, sp4)) for x in range(0, sp4, 640)]
        seg4.append((sp4, L4G))

        def mo4(lo, hi):
            return ff[0:64, lo // 40:hi // 40, :]
        gather_combine(i4a[:], i4b[:], _ld(w4aa), _ld(w4bb), L4G, seg4, GMAX,
                       mo4, 40 * M, ct_out=ct[:], ct_cols=(40 * M, 41 * M))
        nc.vector.tensor_copy(ff[64:128, :, :], _binner(ct[:], 40))
        wfu_s = sing.tile([128, 64], BF, tag="wfu")
        nc.sync.dma_start(wfu_s[:], wfu_in[:])
        bfu_s = sing.tile([64, 1], F32, tag="bfu")
        nc.sync.dma_start(bfu_s[:], bfu_in[:])
        x0p4 = xp4.tile([128, M, 72], BF, tag="pad4x")
        for (s0, nin) in ch4:
            psf = psc.tile([128, 512], F32, tag="ps")
            nc.tensor.matmul(psf[0:64, 0:nin * 40], wfu_s[:],
                             ff[:, s0:s0 + nin, :], start=True, stop=True)
            nc.scalar.activation(out=x0p4[0:64, s0:s0 + nin, PAD:PAD + 40],
                                 in_=psf[0:64, 0:nin * 40], func=AF.Identity,
                                 bias=bfu_s[:], scale=1.0)
        nc.sync.dma_start(x0p4[64:66, :, PAD:PAD + 40], cin4[:])
        wraps(x0p4, 66, 40)

        # interleaved snake execution
        gI = snake_layers("sI", x0p4, 40, 72, ch4, True)
        init_steps = [lambda g=gI: next(g, None) for _ in range(8)]
        init_steps.append(lambda: snake_tail("sI", 40, ch4, None, d4_out,
                                             off=0))
        ii = [0]

        def drip():
            if ii[0] < len(init_steps):
                init_steps[ii[0]]()
                ii[0] += 1

        g1 = snake_layers("s1", x0p1, 128, 160, chE, False)
        mainstep = [0]

        def step_main(fn):
            fn()
            mainstep[0] += 1
            if mainstep[0] % 3 == 2:
                drip()

        for _ in range(8):
            step_main(lambda: next(g1, None))
        pyd = []
        for r in range(2):
            pydr = dram.tile([2, NPTS], F32, tag=f"pyd{r}", name=f"pyd{r}")
            pyd.append(pydr)
        step_main(lambda: snake_tail("s1", 128, chE, base1.ap(), py_out[0],
                                     stage=pyd[0]))
        for r in (1, 2):
            x0p = xpads.tile([128, M, 160], BF, tag="padx")
            step_main(lambda x=x0p, rr=r: coord_round(pyd[rr - 1][:], x, rr))
            g = snake_layers(f"s{r + 1}", x0p, 128, 160, chE, False)
            for _ in range(8):
                step_main(lambda gg=g: next(gg, None))
            step_main(lambda rr=r: snake_tail(
                f"s{r + 1}", 128, chE, pyd[rr - 1][:], py_out[rr],
                stage=(pyd[rr] if rr < 2 else None)))
        while ii[0] < len(init_steps):
            drip()

    nc.finalize()
    return nc


# ============================================================================
# host side
# ============================================================================

def _gather_aids(cx, cy, basew, dmask):
    ix = cx - 0.5
    iy = cy - 0.5
    x0 = np.floor(ix)
    y0 = np.floor(iy)
    wx = ix - x0
    wy = iy - y0
    vx0 = ((x0 >= 0) & (x0 <= 127)).astype(np.float32)
    vx1 = (x0 <= 126).astype(np.float32)
    vy0 = ((y0 >= 0) & (y0 <= 127)).astype(np.float32)
    vy1 = (y0 <= 126).astype(np.float32)
    w00 = (1 - wx) * (1 - wy) * vx0 * vy0 * dmask
    w01 = wx * (1 - wy) * vx1 * vy0 * dmask
    w10 = (1 - wx) * wy * vx0 * vy1 * dmask
    w11 = wx * wy * vx1 * vy1 * dmask
    lidx = (y0 * 128 + x0).astype(np.int64)
    idx0 = np.clip(lidx, 0, IMGROWS - 1) + basew
    idx1 = np.clip(lidx + 128, 0, IMGROWS - 1) + basew
    return idx0, idx1, w00, w01, w10, w11


def _wrap_idx(idx):
    L = idx.shape[0]
    w = idx.astype(np.int16).reshape(L // 16, 16).T
    return np.ascontiguousarray(np.tile(w, (8, 1)))


def _w_tensor(wa, wb):
    t = np.empty((128, wa.shape[0]), np.float32)
    t[0:64] = wa[None, :]
    t[64:128] = wb[None, :]
    return t.astype(BF16)


def _pack_snake(p, sn, d):
    ws = [p['head']['w']] + [p['res'][i]['w'] for i in range(7)]
    bs = [p['head']['b']] + [p['res'][i]['b'] for i in range(7)]
    bns = [p['head_bn']] + list(p['res_bn'])
    for l in range(8):
        w = np.asarray(ws[l], np.float32)
        d[f'{sn}_wc{l}'] = np.ascontiguousarray(
            w.transpose(1, 2, 0)).astype(BF16)
        d[f'{sn}_ws{l}'] = np.ascontiguousarray(w.sum(2).T).astype(BF16)
        d[f'{sn}_cb{l}'] = np.asarray(bs[l], np.float32).reshape(128, 1)
        d[f'{sn}_bn{l}'] = np.ascontiguousarray(np.stack(
            [np.asarray(bns[l]['g'], np.float32),
             np.asarray(bns[l]['b'], np.float32)], 1))
    wf = np.asarray(p['fusion']['w'], np.float32)
    d[f'{sn}_wf'] = np.ascontiguousarray(
        wf.T.reshape(8, 128, 256).transpose(1, 0, 2)).astype(BF16)
    d[f'{sn}_bf'] = np.ascontiguousarray(
        np.asarray(p['fusion']['b'], np.float32).reshape(2, 128).T)
    w0 = np.asarray(p['pred0']['w'], np.float32)
    d[f'{sn}_w0'] = np.ascontiguousarray(
        w0.T.reshape(10, 128, 256).transpose(1, 0, 2)).astype(BF16)
    d[f'{sn}_b0'] = np.ascontiguousarray(
        np.asarray(p['pred0']['b'], np.float32).reshape(2, 128).T)
    w1 = np.asarray(p['pred1']['w'], np.float32)
    d[f'{sn}_w1'] = np.ascontiguousarray(
        w1.T.reshape(2, 128, 64).transpose(1, 0, 2)).astype(BF16)
    d[f'{sn}_b1'] = np.asarray(p['pred1']['b'], np.float32).reshape(64, 1)
    w2 = np.asarray(p['pred2']['w'], np.float32)
    d[f'{sn}_w2'] = np.ascontiguousarray(w2.T).astype(BF16)
    d[f'{sn}_b2'] = np.asarray(p['pred2']['b'], np.float32).reshape(2, 1)


def prepare(cnn_feature, i_it_4py, c_it_4py, i_it_py, c_it_py, ind, params):
    cnn = np.asarray(cnn_feature, np.float32)
    i4 = np.asarray(i_it_4py, np.float32)
    c4 = np.asarray(c_it_4py, np.float32)
    ip = np.asarray(i_it_py, np.float32)
    cp = np.asarray(c_it_py, np.float32)
    ind = np.asarray(ind, np.int32)
    N = ind.shape[0]

    counts = np.bincount(ind, minlength=16)
    order = np.argsort(-counts, kind='stable')
    pairs = [(int(order[i]), int(order[15 - i])) for i in range(8)]
    M = int(max(4, -(-max(int(counts[a] + counts[b]) for a, b in pairs) // 4) * 4))
    NPTS = 128 * M
    NP4 = 40 * M
    L4G = -(-(41 * M) // 128) * 128

    key = (M, LOCAL_BN)
    if key not in _cache:
        _cache[key] = _build(M, LOCAL_BN)
    nc = _cache[key]

    # shared (replicated) tensors
    shared = {}
    _pack_snake(params['init_gcn'], 'sI', shared)
    _pack_snake(params['evolve_gcn'], 's1', shared)
    _pack_snake(params['evolve0'], 's2', shared)
    _pack_snake(params['evolve1'], 's3', shared)
    shared['wfu'] = np.ascontiguousarray(
        np.asarray(params['fuse']['w'], np.float32).T).astype(BF16)
    shared['bfu'] = np.asarray(params['fuse']['b'], np.float32).reshape(64, 1)

    # per-image pair tables
    ptabs = []
    for b in range(16):
        flat = np.ascontiguousarray(
            cnn[b].transpose(1, 2, 0).reshape(IMGROWS, 64))
        nxt = np.vstack([flat[1:], np.zeros((1, 64), np.float32)])
        ptabs.append(np.hstack([flat, nxt]).astype(BF16))
    zrow = np.zeros((1, 128), BF16)

    in_maps = []
    core_ids = []
    for k, (ia, ib) in enumerate(pairs):
        ids = np.concatenate([np.where(ind == ia)[0], np.where(ind == ib)[0]])
        cnt = ids.shape[0]
        core_ids.append(ids)
        m = {}
        m.update(shared)
        m['tab'] = np.ascontiguousarray(
            np.vstack([ptabs[ia], ptabs[ib], zrow]))
        basei = np.where(ind[ids] == ia, 0, IMGROWS).astype(np.int64)
        dmaski = np.ones(cnt, np.float32)

        # round-1 aids (sampling coords = i_it_py)
        cx = np.zeros(NPTS, np.float32)
        cy = np.zeros(NPTS, np.float32)
        bw = np.zeros(NPTS, np.int64)
        dm = np.zeros(NPTS, np.float32)
        cx[:cnt * 128] = ip[ids, :, 0].ravel()
        cy[:cnt * 128] = ip[ids, :, 1].ravel()
        bw[:cnt * 128] = np.repeat(basei, 128)
        dm[:cnt * 128] = np.repeat(dmaski, 128)
        idx0, idx1, w00, w01, w10, w11 = _gather_aids(cx, cy, bw, dm)
        m['idx1a'] = _wrap_idx(idx0)
        m['idx1b'] = _wrap_idx(idx1)
        m['w1aa'] = _w_tensor(w00, w01)
        m['w1bb'] = _w_tensor(w10, w11)

        # init aids: samples + centers
        cx = np.zeros(L4G, np.float32)
        cy = np.zeros(L4G, np.float32)
        bw = np.zeros(L4G, np.int64)
        dm = np.zeros(L4G, np.float32)
        cx[:cnt * 40] = i4[ids, :, 0].ravel()
        cy[:cnt * 40] = i4[ids, :, 1].ravel()
        bw[:cnt * 40] = np.repeat(basei, 40)
        dm[:cnt * 40] = np.repeat(dmaski, 40)
        cent = 0.5 * (i4[ids].min(1) + i4[ids].max(1))
        cx[40 * M:40 * M + cnt] = cent[:, 0]
        cy[40 * M:40 * M + cnt] = cent[:, 1]
        bw[40 * M:40 * M + cnt] = basei
        dm[40 * M:40 * M + cnt] = dmaski
        idx0, idx1, w00, w01, w10, w11 = _gather_aids(cx, cy, bw, dm)
        m['idx4a'] = _wrap_idx(idx0)
        m['idx4b'] = _wrap_idx(idx1)
        m['w4aa'] = _w_tensor(w00, w01)
        m['w4bb'] = _w_tensor(w10, w11)

        cin1 = np.zeros((2, NPTS), np.float32)
        cin1[:, :cnt * 128] = (cp[ids] * RO).reshape(cnt * 128, 2).T
        m['cin1'] = cin1.astype(BF16)
        cin4 = np.zeros((2, NP4), np.float32)
        cin4[:, :cnt * 40] = c4[ids].reshape(cnt * 40, 2).T
        m['cin4'] = cin4.astype(BF16)
        b1 = np.zeros((2, NPTS), np.float32)
        b1[:, :cnt * 128] = (ip[ids] * RO).reshape(cnt * 128, 2).T
        m['base1'] = b1

        dmt = np.zeros((128, M), np.float32)
        dmt[:, :cnt] = 1.0
        m['dmask_t'] = dmt
        sbt = np.zeros((128, M), np.float32)
        sbt[:, :cnt] = basei[None, :].astype(np.float32)
        m['sbase_t'] = sbt
        ls = np.zeros((128, 2), np.float32)
        ls[:, 0] = 1.0 / (cnt * 128)
        ls[:, 1] = 1.0 / (cnt * 40)
        m['locstat'] = ls
        ld = np.zeros((128, 2), np.float32)
        ld[:, 0] = float((M - cnt) * 128)
        ld[:, 1] = float((M - cnt) * 40)
        m['locdum'] = ld
        in_maps.append(m)

    return nc, in_maps, core_ids, M, i4, N


def assemble(results, core_ids, i4, N):
    ex_pred = np.zeros((N, 4, 2), np.float32)
    pys = np.zeros((3, N, 128, 2), np.float32)
    for k in range(NCORES):
        ids = core_ids[k]
        cnt = ids.shape[0]
        d4 = results[k]['d4']
        ipoly = i4[ids] + d4[:, :cnt * 40].T.reshape(cnt, 40, 2)
        ex_pred[ids] = ipoly[:, ::10, :]
        for r in range(3):
            pr = results[k][f'py{r + 1}']
            pys[r, ids] = pr[:, :cnt * 128].T.reshape(cnt, 128, 2)
    return ex_pred, pys


def kernel(**inputs):
    nc, in_maps, core_ids, M, i4, N = prepare(**inputs)
    res = run_bass_kernel_spmd(nc, in_maps, core_ids=list(range(NCORES)))
    return assemble(res.results, core_ids, i4, N)
